# revision 1
# baseline (speedup 1.0000x reference)
"""LID detector kernel for Trainium2 (8 NeuronCores, data-parallel over batch).

Per core (batch shard of 32):
  - mean-pool each feature map over space -> q [C, 32] (transposed layout)
  - -d2 = 2*q.r - ||r||^2 - ||q||^2 via PE matmuls into PSUM, evicted into a
    stacked [128, 2000] buffer (partition quadrant = layer)
  - top-24 smallest d2 via 3 rounds of DVE max8 + match_replace
  - LID = -2k / (sum_{i=1..20} ln d2_i - 20 ln d2_20)  (no sqrt needed)
  - logit = w . lid + b -> sigmoid -> out [32]
"""

import sys

for _p in ("/opt/trn_rl_repo", "/root/.axon_site/_ro/trn_rl_repo"):
    if _p not in sys.path:
        sys.path.append(_p)

import ml_dtypes
import numpy as np

import concourse.mybir as mybir
from concourse import bass, bacc
from concourse.tile import TileContext
from concourse.bass_utils import run_bass_kernel_spmd

F32 = mybir.dt.float32
BF16 = mybir.dt.bfloat16
N_CORES = 8
B = 32  # batch shard per core
R = 2000
K = 20
LAYERS = [(64, 3136), (128, 784), (256, 196), (512, 49)]  # (C, H*W)
NEG_BIG = -3.0e38

# column j of qT holds sample SIGMA[j] of the local shard
SIGMA = np.array([2 * j for j in range(16)] + [2 * j + 1 for j in range(16)])


def build_nc():
    nc = bacc.Bacc("TRN2", target_bir_lowering=False, debug=False,
                   num_devices=N_CORES)

    feats = [nc.dram_tensor(f"feat{l}", [B, C, HW], BF16, kind="ExternalInput")
             for l, (C, HW) in enumerate(LAYERS)]
    refTs = [nc.dram_tensor(f"refT{l}", [C, R], F32, kind="ExternalInput")
             for l, (C, _) in enumerate(LAYERS)]
    regw = nc.dram_tensor("regw", [1, 4], F32, kind="ExternalInput")
    regb = nc.dram_tensor("regb", [1, 1], F32, kind="ExternalInput")
    out = nc.dram_tensor("out", [B, 1], F32, kind="ExternalOutput")
    import os
    _dbg = os.environ.get("DEBUG_LID") == "1"
    if _dbg:
        dbg_lid = nc.dram_tensor("dbg_lid", [128, 1], F32, kind="ExternalOutput")
        dbg_vals = nc.dram_tensor("dbg_vals", [128, 24], F32, kind="ExternalOutput")
        dbg_q = nc.dram_tensor("dbg_q", [64, B], F32, kind="ExternalOutput")
        dbg_tk = nc.dram_tensor("dbg_tk", [128, R], F32, kind="ExternalOutput")
        dbg_rn2a = nc.dram_tensor("dbg_rn2a", [65, R], F32, kind="ExternalOutput")
        dbg_rn2b = nc.dram_tensor("dbg_rn2b", [1, R], F32, kind="ExternalOutput")

    with TileContext(nc) as tc:
        with (
            tc.tile_pool(name="persist", bufs=1) as pp,
            tc.tile_pool(name="ft", bufs=6) as fp,
            tc.tile_pool(name="sq", bufs=2) as sqp,
        ):
            # ---- persistent tiles
            rt = {}   # (l, i) -> refT chunk tile [Cc, R]
            for l, (C, _) in enumerate(LAYERS):
                for i in range(0, C, 128):
                    Cc = min(128, C - i)
                    rt[(l, i)] = pp.tile([Cc, R], F32, tag=f"rt{l}_{i}",
                                         name=f"rt{l}_{i}")
            act_scratch = pp.tile([128, 3136], BF16, tag="act_scratch",
                                  name="act_scratch")
            rn2a = pp.tile([65, R], F32, tag="rn2a", name="rn2a")
            rn2b = pp.tile([1, R], F32, tag="rn2b", name="rn2b")
            rn2base = [(rn2a, 0), (rn2a, 32), (rn2a, 64), (rn2b, 0)]
            rn2row = [t[b:b + 1, :] for (t, b) in rn2base]
            qT = {}
            for l, (C, _) in enumerate(LAYERS):
                for i in range(0, C, 128):
                    Cc = min(128, C - i)
                    qT[(l, i)] = pp.tile([Cc, B], F32, tag=f"qT{l}_{i}", name=f"qT{l}_{i}")
            qn2neg = [pp.tile([B, 1], F32, tag=f"qn2_{l}", name=f"qn2_{l}") for l in range(4)]
            topkbuf = pp.tile([128, R], F32, tag="topkbuf", name="topkbuf")
            vals = pp.tile([128, 24], F32, tag="vals", name="vals")
            ones_col = pp.tile([128, 1], F32, tag="ones_col", name="ones_col")
            ones_row = pp.tile([1, B], F32, tag="ones_row", name="ones_row")
            negones_all = pp.tile([65, B], F32, tag="negones_all", name="negones_all")
            wb_sb = pp.tile([1, 5], F32, tag="wb_sb", name="wb_sb")
            tmp0 = pp.tile([128, 16], F32, tag="tmp0", name="tmp0")

            nc.vector.memset(ones_col[:], 1.0)
            nc.vector.memset(ones_row[:], 1.0)
            nc.vector.memset(negones_all[:], -1.0)
            nc.sync.dma_start(out=wb_sb[0:1, 0:4], in_=regw[:])
            nc.sync.dma_start(out=wb_sb[0:1, 4:5], in_=regb[:])

            # ---- ref squared norms: rn2[l] [1, R] = sum_c refT^2
            with tc.tile_pool(name="psumA", bufs=1,
                              space=bass.MemorySpace.PSUM) as pA:
                for l, (C, _) in enumerate(LAYERS):
                    ps = pA.tile([1, R], F32, tag="rn2ps", name="rn2ps")
                    chunks = list(range(0, C, 128))
                    for ci, i in enumerate(chunks):
                        Cc = min(128, C - i)
                        sq = sqp.tile([128, R], F32, tag="sq", name="sq")
                        nc.scalar.square(sq[0:Cc, :], rt[(l, i)][:])
                        for c0 in range(0, R, 512):
                            n = min(512, R - c0)
                            nc.tensor.matmul(
                                ps[0:1, c0:c0 + n],
                                ones_col[0:Cc, 0:1],
                                sq[0:Cc, c0:c0 + n],
                                start=(ci == 0), stop=(ci == len(chunks) - 1),
                            )
                    nc.scalar.copy(rn2row[l], ps[:])

            # ---- pooling: fill qT columns (sample order SIGMA)
            # layer 0: C=64, 2 samples per 128 partitions
            C, HW = LAYERS[0]
            for t in range(8):
                tile = fp.tile([128, 2, HW], BF16, tag="ft", name="ft")
                src = bass.AP(feats[0], 4 * t * C * HW,
                              [[HW, 128], [2 * C * HW, 2], [1, HW]])
                nc.sync.dma_start(out=tile[:], in_=src)
                if t < 4:
                    nc.vector.tensor_reduce(
                        tmp0[:, 2 * t:2 * t + 2], tile[:],
                        axis=mybir.AxisListType.X, op=mybir.AluOpType.add)
                else:
                    for g in range(2):
                        nc.scalar.activation(
                            act_scratch[:, 0:HW], tile[:, g, :],
                            mybir.ActivationFunctionType.Copy,
                            accum_out=tmp0[:, 2 * t + g:2 * t + g + 1])
            nc.vector.tensor_copy(qT[(0, 0)][:, 0:16], tmp0[0:64, :])
            nc.vector.tensor_copy(qT[(0, 0)][:, 16:32], tmp0[64:128, :])

            # layers 1..3: per chunk, DMA samples with stride 2 (even then odd)
            for l in (1, 2, 3):
                C, HW = LAYERS[l]
                n_chunks = C // 128
                g = B // (2 * 4 // 1)  # placeholder, set below
                # samples per DMA: L1:4 (8 DMAs), L2:8 (4 DMAs), L3:16 (2 DMAs)
                spd = {1: 4, 2: 8, 3: 16}[l]
                ndma = B // spd
                for i in range(n_chunks):
                    for t in range(ndma):
                        # cols spd*t .. spd*t+spd-1 -> samples SIGMA[col]
                        # = base + 2*j, base = 2*spd*t if even half else ...
                        col0 = spd * t
                        s_base = int(SIGMA[col0])
                        tile = fp.tile([128, spd, HW], BF16, tag="ft", name="ft")
                        src = bass.AP(
                            feats[l],
                            s_base * C * HW + 128 * i * HW,
                            [[HW, 128], [2 * C * HW, spd], [1, HW]])
                        nc.sync.dma_start(out=tile[:], in_=src)
                        if l == 1:
                            for g in range(spd):
                                nc.scalar.activation(
                                    act_scratch[:, 0:HW], tile[:, g, :],
                                    mybir.ActivationFunctionType.Copy,
                                    accum_out=qT[(l, 128 * i)][:, col0 + g:col0 + g + 1])
                        else:
                            nc.vector.tensor_reduce(
                                qT[(l, 128 * i)][:, col0:col0 + spd], tile[:],
                                axis=mybir.AxisListType.X, op=mybir.AluOpType.add)

            for l, (C, _) in enumerate(LAYERS):
                for i in range(0, C, 128):
                    Cc = min(128, C - i)
                    nc.sync.dma_start(out=rt[(l, i)][:],
                                      in_=refTs[l][i:i + Cc, :])

            # ---- scale qT by 2/HW (so lhsT holds 2*q), qn2neg
            with tc.tile_pool(name="psumB", bufs=1,
                              space=bass.MemorySpace.PSUM) as pB:
                for l, (C, HW) in enumerate(LAYERS):
                    chunks = list(range(0, C, 128))
                    qps = pB.tile([B, 1], F32, tag="qn2ps", name="qn2ps")
                    for ci, i in enumerate(chunks):
                        Cc = min(128, C - i)
                        nc.scalar.mul(qT[(l, i)][:], qT[(l, i)][:], 2.0 / HW)
                        qsq = sqp.tile([128, B], F32, tag="qsq", name="qsq")
                        # (2q * 0.5)^2 = q^2
                        nc.scalar.activation(
                            qsq[0:Cc, :], qT[(l, i)][:],
                            mybir.ActivationFunctionType.Square, scale=0.5)
                        nc.tensor.matmul(
                            qps[:], qsq[0:Cc, :], ones_col[0:Cc, 0:1],
                            start=(ci == 0), stop=(ci == len(chunks) - 1))
                    nc.scalar.mul(qn2neg[l][:], qps[:], -1.0)

                # ---- distances: psum = 2q.r - rn2 ; evict + qn2neg -> -d2
                for l, (C, _) in enumerate(LAYERS):
                    chunks = list(range(0, C, 128))
                    for c0 in range(0, R, 512):
                        n = min(512, R - c0)
                        dps = pB.tile([B, 512], F32, tag="d2ps", name="d2ps")
                        for ci, i in enumerate(chunks):
                            Cc = min(128, C - i)
                            nc.tensor.matmul(
                                dps[:, 0:n], qT[(l, i)][:],
                                rt[(l, i)][:, c0:c0 + n],
                                start=(ci == 0), stop=False)
                        rn2t, rn2b_ = rn2base[l]
                        nc.tensor.matmul(
                            dps[:, 0:n], negones_all[rn2b_:rn2b_ + 1, :],
                            rn2t[rn2b_:rn2b_ + 1, c0:c0 + n],
                            start=False, stop=True)
                        nc.vector.tensor_scalar(
                            topkbuf[32 * l:32 * l + 32, c0:c0 + n],
                            dps[:, 0:n], qn2neg[l][:], None,
                            op0=mybir.AluOpType.add)

                if _dbg:
                    nc.sync.dma_start(out=dbg_tk[:], in_=topkbuf[:])
                    nc.sync.dma_start(out=dbg_rn2a[:], in_=rn2a[:])
                    nc.sync.dma_start(out=dbg_rn2b[:], in_=rn2b[:])
                # ---- top-24 (ascending d2 == descending -d2)
                nc.vector.max(vals[:, 0:8], topkbuf[:])
                nc.vector.match_replace(topkbuf[:], vals[:, 0:8], topkbuf[:],
                                        NEG_BIG)
                nc.vector.max(vals[:, 8:16], topkbuf[:])
                nc.vector.match_replace(topkbuf[:], vals[:, 8:16], topkbuf[:],
                                        NEG_BIG)
                nc.vector.max(vals[:, 16:24], topkbuf[:])

                # ---- LID
                ln2 = pp.tile([128, 24], F32, tag="ln2", name="ln2")
                S = pp.tile([128, 1], F32, tag="S", name="S")
                denom = pp.tile([128, 1], F32, tag="denom", name="denom")
                lid = pp.tile([128, 1], F32, tag="lid", name="lid")
                # clamp: vals <= -1e-30 so that -vals >= 1e-30
                nc.vector.tensor_scalar_min(vals[:], vals[:], -1e-30)
                nc.scalar.activation(ln2[:], vals[:],
                                     mybir.ActivationFunctionType.Ln,
                                     scale=-1.0)
                nc.vector.tensor_reduce(S[:], ln2[:, 1:21],
                                        axis=mybir.AxisListType.X,
                                        op=mybir.AluOpType.add)
                # denom = -20*ln2[20] + S  (= sum ln d2_i - 20 ln d2_20)
                nc.vector.tensor_scalar(denom[:], ln2[:, 20:21], -20.0, S[:],
                                        op0=mybir.AluOpType.mult,
                                        op1=mybir.AluOpType.add)
                nc.vector.reciprocal(lid[:], denom[:])
                nc.vector.tensor_scalar_mul(lid[:], lid[:], -2.0 * K)
                if _dbg:
                    nc.sync.dma_start(out=dbg_lid[:], in_=lid[:])
                    nc.sync.dma_start(out=dbg_vals[:], in_=vals[:])
                    nc.sync.dma_start(out=dbg_q[:], in_=qT[(0, 0)][:])

                # ---- regression + sigmoid
                lid4 = pp.tile([B, 4], F32, tag="lid4", name="lid4")
                for l in range(4):
                    nc.vector.tensor_copy(lid4[:, l:l + 1],
                                          lid[32 * l:32 * l + 32, :])
                wps = pB.tile([B, 5], F32, tag="wps", name="wps")
                nc.tensor.matmul(wps[:], ones_row[:], wb_sb[:],
                                 start=True, stop=True)
                wbc = pp.tile([B, 5], F32, tag="wbc", name="wbc")
                nc.scalar.copy(wbc[:], wps[:])
                prod = pp.tile([B, 4], F32, tag="prod", name="prod")
                nc.vector.tensor_tensor(prod[:], lid4[:], wbc[:, 0:4],
                                        op=mybir.AluOpType.mult)
                ssum = pp.tile([B, 1], F32, tag="ssum", name="ssum")
                nc.vector.tensor_reduce(ssum[:], prod[:],
                                        axis=mybir.AxisListType.X,
                                        op=mybir.AluOpType.add)
                res = pp.tile([B, 1], F32, tag="res", name="res")
                nc.scalar.activation(res[:], ssum[:],
                                     mybir.ActivationFunctionType.Sigmoid,
                                     bias=wbc[:, 4:5])
                nc.sync.dma_start(out=out[:], in_=res[:])

    nc.compile()
    return nc


_NC = None


def _get_nc():
    global _NC
    if _NC is None:
        _NC = build_nc()
    return _NC


def run(trace=False, **inputs):
    nc = _get_nc()
    feats = [np.asarray(inputs[f"feat{l}"], dtype=np.float32) for l in range(4)]
    refTs = [np.ascontiguousarray(np.asarray(inputs[f"ref{l}"],
                                             dtype=np.float32).T)
             for l in range(4)]
    regw = np.asarray(inputs["reg_w"], dtype=np.float32).reshape(1, 4)
    regb = np.asarray(inputs["reg_b"], dtype=np.float32).reshape(1, 1)
    assert int(inputs.get("k", K)) == K

    in_maps = []
    for c in range(N_CORES):
        m = {}
        for l, (C, HW) in enumerate(LAYERS):
            m[f"feat{l}"] = np.ascontiguousarray(
                feats[l][c * B:(c + 1) * B].reshape(B, C, HW)).astype(
                    ml_dtypes.bfloat16)
            m[f"refT{l}"] = refTs[l]
        m["regw"] = regw
        m["regb"] = regb
        in_maps.append(m)

    res = run_bass_kernel_spmd(nc, in_maps, core_ids=list(range(N_CORES)),
                               trace=trace)
    full = np.empty((N_CORES * B,), dtype=np.float32)
    for c in range(N_CORES):
        shard = np.empty((B,), dtype=np.float32)
        shard[SIGMA] = res.results[c]["out"][:, 0]
        full[c * B:(c + 1) * B] = shard
    return full, res


def kernel(**inputs):
    return run(trace=False, **inputs)[0]



# revision 13
# speedup vs baseline: 2.0927x; 2.0927x over previous
"""LID detector kernel for Trainium2 (8 NeuronCores, data-parallel over batch).

Per core (batch shard of 32), all heavy data in fp8-e3m4 with per-layer
scaling alpha_l = 2^(5-l):
  - L0 pooling on PE (ones-matmul over hw-major layout, psum accumulate)
  - L1 pooling on Act (activation Copy + accum_out per sample; 8 samples on PE)
  - L2/L3 pooling on DVE (tensor_reduce axis=X over c-major layout)
  - s' = alpha*(2 q.r - rn2) accumulated in 4 psum banks [128, 512]
    (partition = 4 layers x 32 samples); rn2 hi/lo enters via one masked
    bf16 K=8 matmul per bank; alpha^2*4*qn2 rides in a pad column of bank 3
  - top-24 of s' per bank directly on psum (DVE max8/match_replace), then
    a 96 -> 24 merge
  - ln(alpha*d2) = Ln(-(s' - alpha*qn2)); LID = -2k/(sum ln - 20 ln_k)
    (alpha cancels); logit via one matmul against a host-built selector
    wsel[p, b] = w[p//32] * (p%32 == b); sigmoid with bias b.
"""

import sys

for _p in ("/opt/trn_rl_repo", "/root/.axon_site/_ro/trn_rl_repo"):
    if _p not in sys.path:
        sys.path.append(_p)

import ml_dtypes
import numpy as np

import concourse.mybir as mybir
from concourse import bass, bacc
from concourse.tile import TileContext
from concourse.bass_utils import run_bass_kernel_spmd

F32 = mybir.dt.float32
BF16 = mybir.dt.bfloat16
F8 = mybir.dt.float8e3  # e3m4
F8NP = ml_dtypes.float8_e3m4
N_CORES = 8
B = 32
R = 2000
RP = 2048
K = 20
NEG_BIG = -3.0e38
QN2_SHIFT = 65536.0
QN2_COL = 504          # local col in bank 3 (global 2040)
LOGIT_COL = 470        # local cols 470:502 in bank 3

LAYERS = [(64, 3136), (128, 784), (256, 196), (512, 49)]  # (C, HW)
ALPHA = [32.0, 16.0, 8.0, 4.0]
ACT_L1 = 24            # samples of L1 pooled on Act; rest on PE
HWP0, NCH0 = 3200, 25  # L0 hw padded, hw chunks
HWP1, NCH1 = 896, 7    # L1-PE hw padded, hw chunks
CB0 = 64 * B           # 2048 = L0 (c,b) free space
CB1 = 128 * (B - ACT_L1)  # 1024


def build_nc():
    nc = bacc.Bacc("TRN2", target_bir_lowering=False, debug=False,
                   num_devices=N_CORES)

    ft0 = nc.dram_tensor("ft0", [128, NCH0, CB0], F8, kind="ExternalInput")
    ft1pe = nc.dram_tensor("ft1pe", [128, NCH1, CB1], F8, kind="ExternalInput")
    f1a = nc.dram_tensor("f1a", [128, ACT_L1, 784], F8, kind="ExternalInput")
    f2 = nc.dram_tensor("f2", [128, 2, 32 * 196], F8, kind="ExternalInput")
    f3 = nc.dram_tensor("f3", [128, 4, 32 * 49], F8, kind="ExternalInput")
    rts = [nc.dram_tensor(f"rt{l}", [C, RP], F8, kind="ExternalInput")
           for l, (C, _) in enumerate(LAYERS)]
    rn2r = nc.dram_tensor("rn2r", [8, RP], BF16, kind="ExternalInput")
    mask8 = nc.dram_tensor("mask8", [8, 128], BF16, kind="ExternalInput")
    ainv = nc.dram_tensor("ainv", [128, 1], F32, kind="ExternalInput")
    wsel = nc.dram_tensor("wsel", [128, B], F32, kind="ExternalInput")
    regb = nc.dram_tensor("regb", [1, 1], F32, kind="ExternalInput")
    out = nc.dram_tensor("out", [1, B], F32, kind="ExternalOutput")

    with TileContext(nc) as tc:
        with (
            tc.tile_pool(name="persist", bufs=1) as pp,
            tc.tile_pool(name="pd", bufs=4, space=bass.MemorySpace.PSUM) as pdp,
            tc.tile_pool(name="lp", bufs=4, space=bass.MemorySpace.PSUM) as lpp,
        ):
            # ---- SBUF tiles
            ft0sb = pp.tile([128, NCH0, CB0], F8, tag="ft0sb", name="ft0sb")
            ft1sb = pp.tile([128, NCH1, CB1], F8, tag="ft1sb", name="ft1sb")
            f1asb = pp.tile([128, ACT_L1, 784], F8, tag="f1asb", name="f1asb")
            f2sb = [pp.tile([128, 32, 196], F8, tag=f"f2sb{i}", name=f"f2sb{i}")
                    for i in range(2)]
            f3sb = [pp.tile([128, 32, 49], F8, tag=f"f3sb{i}", name=f"f3sb{i}")
                    for i in range(4)]
            rtsb = {}
            for l, (C, _) in enumerate(LAYERS):
                for ci in range(C // 128 if C >= 128 else 1):
                    Cc = min(128, C)
                    rtsb[(l, ci)] = pp.tile([Cc, RP], F8, tag=f"rt{l}_{ci}",
                                            name=f"rt{l}_{ci}")
            rn2sb = pp.tile([8, RP], BF16, tag="rn2sb", name="rn2sb")
            m8sb = pp.tile([8, 128], BF16, tag="m8sb", name="m8sb")
            ainvsb = pp.tile([128, 1], F32, tag="ainvsb", name="ainvsb")
            wselsb = pp.tile([128, B], F32, tag="wselsb", name="wselsb")
            ones8 = pp.tile([128, 1], F8, tag="ones8", name="ones8")
            onesc = pp.tile([128, 1], F32, tag="onesc", name="onesc")
            scr = pp.tile([128, 784], F8, tag="scr", name="scr")
            ev0 = pp.tile([1, CB0], F32, tag="ev0", name="ev0")
            ev1 = pp.tile([1, CB1], F32, tag="ev1", name="ev1")
            q0f = pp.tile([64, B], F32, tag="q0f", name="q0f")
            q1f = pp.tile([128, B], F32, tag="q1f", name="q1f")
            q2f = [pp.tile([128, B], F32, tag=f"q2f{i}", name=f"q2f{i}")
                   for i in range(2)]
            q3f = [pp.tile([128, B], F32, tag=f"q3f{i}", name=f"q3f{i}")
                   for i in range(4)]
            qT = {}
            qT[(0, 0)] = pp.tile([64, B], F8, tag="qT0", name="qT0")
            qT[(1, 0)] = pp.tile([128, B], F8, tag="qT1", name="qT1")
            for i in range(2):
                qT[(2, i)] = pp.tile([128, B], F8, tag=f"qT2_{i}", name=f"qT2_{i}")
            for i in range(4):
                qT[(3, i)] = pp.tile([128, B], F8, tag=f"qT3_{i}", name=f"qT3_{i}")
            qsq = {}
            qsq[(0, 0)] = pp.tile([64, B], F32, tag="qs0", name="qs0")
            qsq[(1, 0)] = pp.tile([128, B], F32, tag="qs1", name="qs1")
            for i in range(2):
                qsq[(2, i)] = pp.tile([128, B], F32, tag=f"qs2_{i}", name=f"qs2_{i}")
            for i in range(4):
                qsq[(3, i)] = pp.tile([128, B], F32, tag=f"qs3_{i}", name=f"qs3_{i}")
            qn2sb = pp.tile([128, 1], F32, tag="qn2sb", name="qn2sb")
            vals96 = pp.tile([128, 96], F32, tag="vals96", name="vals96")
            mvals = pp.tile([128, 24], F32, tag="mvals", name="mvals")
            negd2 = pp.tile([128, 24], F32, tag="negd2", name="negd2")
            ln2t = pp.tile([128, 24], F32, tag="ln2t", name="ln2t")
            St = pp.tile([128, 1], F32, tag="St", name="St")
            den = pp.tile([128, 1], F32, tag="den", name="den")
            lid = pp.tile([128, 1], F32, tag="lid", name="lid")
            res = pp.tile([1, B], F32, tag="res", name="res")
            bcol = pp.tile([1, 1], F32, tag="bcol", name="bcol")

            nc.vector.memset(ones8[:], 1.0)
            nc.vector.memset(onesc[:], 1.0)

            # ---- DMA stream (SP queue), interleaved by consumer
            nc.sync.dma_start(out=m8sb[:], in_=mask8[:])
            nc.sync.dma_start(out=rn2sb[:], in_=rn2r[:])
            nc.sync.dma_start(out=ainvsb[:], in_=ainv[:])
            nc.sync.dma_start(out=wselsb[:], in_=wsel[:])
            nc.sync.dma_start(out=bcol[:], in_=regb[:])
            for i in range(2):
                nc.sync.dma_start(out=f2sb[i][:], in_=f2[:, i, :])
            for i in range(3):
                nc.sync.dma_start(out=f3sb[i][:], in_=f3[:, i, :])

            def ft0_piece(p, n):
                src = bass.AP(ft0, p * CB0,
                              [[NCH0 * CB0, 128], [CB0, n], [1, CB0]])
                nc.sync.dma_start(out=ft0sb[:, p:p + n, :], in_=src)

            def f1a_piece(p, n):
                src = bass.AP(f1a, p * 784,
                              [[ACT_L1 * 784, 128], [784, n], [1, 784]])
                nc.sync.dma_start(out=f1asb[:, p:p + n, :], in_=src)

            ft0_piece(0, 5)
            f1a_piece(0, 8)
            ft0_piece(5, 5)
            nc.sync.dma_start(out=rtsb[(0, 0)][:], in_=rts[0][:])
            nc.sync.dma_start(out=rtsb[(1, 0)][:], in_=rts[1][:])
            f1a_piece(8, 8)
            ft0_piece(10, 5)
            for ci in range(2):
                nc.sync.dma_start(out=rtsb[(2, ci)][:],
                                  in_=rts[2][128 * ci:128 * ci + 128, :])
            f1a_piece(16, 8)
            ft0_piece(15, 5)
            for ci in range(4):
                nc.sync.dma_start(out=rtsb[(3, ci)][:],
                                  in_=rts[3][128 * ci:128 * ci + 128, :])
            ft0_piece(20, 5)
            nc.sync.dma_start(out=ft1sb[:], in_=ft1pe[:])
            nc.sync.dma_start(out=f3sb[3][:], in_=f3[:, 3, :])

            # ---- pooling: L2 / L3 chunks 0-2 on DVE
            for i in range(2):
                nc.vector.tensor_reduce(q2f[i][:], f2sb[i][:],
                                        axis=mybir.AxisListType.X,
                                        op=mybir.AluOpType.add)
            for i in range(3):
                nc.vector.tensor_reduce(q3f[i][:], f3sb[i][:],
                                        axis=mybir.AxisListType.X,
                                        op=mybir.AluOpType.add)

            # ---- pooling: L1 samples 0..ACT_L1 on Act
            for s in range(ACT_L1):
                nc.scalar.activation(scr[:], f1asb[:, s, :],
                                     mybir.ActivationFunctionType.Copy,
                                     accum_out=q1f[:, s:s + 1])

            # ---- pooling: L0 on PE (chunk-major, 4 open psum blocks)
            lp0 = [lpp.tile([1, 512], F32, tag="lp", name=f"lp0_{b}")
                   for b in range(4)]
            for ch in range(NCH0):
                for blk in range(4):
                    nc.tensor.matmul(
                        lp0[blk][:], ones8[:],
                        ft0sb[:, ch, 512 * blk:512 * blk + 512],
                        start=(ch == 0), stop=(ch == NCH0 - 1))
            for blk in range(4):
                nc.scalar.copy(ev0[0:1, 512 * blk:512 * blk + 512],
                               lp0[blk][:])
            # scatter [1, 2048] -> [64, 32] via SWDGE
            nc.gpsimd.dma_start(out=q0f[:], in_=ev0[0:1, :])

            # ---- pooling: L1 samples ACT_L1.. on PE
            lp1 = [lpp.tile([1, 512], F32, tag="lp", name=f"lp1_{b}")
                   for b in range(2)]
            for ch in range(NCH1):
                for blk in range(2):
                    nc.tensor.matmul(
                        lp1[blk][:], ones8[:],
                        ft1sb[:, ch, 512 * blk:512 * blk + 512],
                        start=(ch == 0), stop=(ch == NCH1 - 1))
            for blk in range(2):
                nc.scalar.copy(ev1[0:1, 512 * blk:512 * blk + 512],
                               lp1[blk][:])
            nc.gpsimd.dma_start(out=q1f[:, ACT_L1:B], in_=ev1[0:1, :])

            # ---- L3 chunk 3 (tail)
            nc.vector.tensor_reduce(q3f[3][:], f3sb[3][:],
                                    axis=mybir.AxisListType.X,
                                    op=mybir.AluOpType.add)

            # ---- scale + cast to fp8 qT' (Pool), squares (Act)
            qsrc = {(0, 0): q0f, (1, 0): q1f, (2, 0): q2f[0], (2, 1): q2f[1],
                    (3, 0): q3f[0], (3, 1): q3f[1], (3, 2): q3f[2],
                    (3, 3): q3f[3]}
            chunks_of = {0: [(0, 0)], 1: [(1, 0)], 2: [(2, 0), (2, 1)],
                         3: [(3, 0), (3, 1), (3, 2), (3, 3)]}
            cc_of = {key: (64 if key[0] == 0 else 128)
                     for keys in chunks_of.values() for key in keys}
            for l, (C, HW) in enumerate(LAYERS):
                sc = 2.0 * ALPHA[l] / HW
                for key in chunks_of[l]:
                    nc.gpsimd.tensor_scalar(qT[key][:], qsrc[key][:],
                                            sc, None,
                                            op0=mybir.AluOpType.mult)
                    nc.scalar.square(qsq[key][:], qT[key][:])

            # ---- distances: 4 banks of [128, 512]
            pd = [pdp.tile([128, 512], F32, tag="d2", name=f"d2_{b}")
                  for b in range(4)]
            for b in range(4):
                c0 = 512 * b
                for l in range(4):
                    keys = chunks_of[l]
                    for ki, key in enumerate(keys):
                        nc.tensor.matmul(
                            pd[b][32 * l:32 * l + 32, :], qT[key][:],
                            rtsb[key][:, c0:c0 + 512],
                            start=(ki == 0), stop=False,
                            tile_position=(0, 32 * l))
                    if b == 3:
                        # qn2 pad column (shifted by -QN2_SHIFT via rn2 row)
                        for key in keys:
                            nc.tensor.matmul(
                                pd[3][32 * l:32 * l + 32, QN2_COL:QN2_COL + 1],
                                qsq[key][:], onesc[0:cc_of[key], 0:1],
                                start=False, stop=False,
                                tile_position=(0, 32 * l))
                # rn2 (+qn2 shift) for all 4 bands, closes the bank group
                nc.tensor.matmul(pd[b][:], m8sb[:],
                                 rn2sb[:, c0:c0 + 512],
                                 start=False, stop=True)

            # ---- qn2 recovery (before match_replace mutates bank 3)
            nc.vector.tensor_scalar(qn2sb[:], pd[3][:, QN2_COL:QN2_COL + 1],
                                    QN2_SHIFT, ainvsb[:],
                                    op0=mybir.AluOpType.add,
                                    op1=mybir.AluOpType.mult)

            # ---- top-24 per bank on psum, then merge 96 -> 24
            for b in range(4):
                v = vals96[:, 24 * b:24 * b + 8]
                nc.vector.max(v, pd[b][:])
                nc.vector.match_replace(pd[b][:], v, pd[b][:], NEG_BIG)
                v = vals96[:, 24 * b + 8:24 * b + 16]
                nc.vector.max(v, pd[b][:])
                nc.vector.match_replace(pd[b][:], v, pd[b][:], NEG_BIG)
                nc.vector.max(vals96[:, 24 * b + 16:24 * b + 24], pd[b][:])
            nc.vector.max(mvals[:, 0:8], vals96[:])
            nc.vector.match_replace(vals96[:], mvals[:, 0:8], vals96[:],
                                    NEG_BIG)
            nc.vector.max(mvals[:, 8:16], vals96[:])
            nc.vector.match_replace(vals96[:], mvals[:, 8:16], vals96[:],
                                    NEG_BIG)
            nc.vector.max(mvals[:, 16:24], vals96[:])

            # ---- LID
            nc.vector.tensor_scalar(negd2[:], mvals[:], qn2sb[:], None,
                                    op0=mybir.AluOpType.subtract)
            nc.vector.tensor_scalar_min(negd2[:], negd2[:], -1e-30)
            nc.scalar.activation(ln2t[:], negd2[:],
                                 mybir.ActivationFunctionType.Ln,
                                 scale=-1.0)
            nc.vector.tensor_reduce(St[:], ln2t[:, 1:K + 1],
                                    axis=mybir.AxisListType.X,
                                    op=mybir.AluOpType.add)
            nc.vector.tensor_scalar(den[:], ln2t[:, K:K + 1], -float(K),
                                    St[:], op0=mybir.AluOpType.mult,
                                    op1=mybir.AluOpType.add)
            nc.vector.reciprocal(lid[:], den[:])
            nc.vector.tensor_scalar_mul(lid[:], lid[:], -2.0 * K)

            # ---- logit + sigmoid
            nc.tensor.matmul(pd[3][0:1, LOGIT_COL:LOGIT_COL + B],
                             lid[:], wselsb[:], start=True, stop=True,
                             skip_group_check=True)
            nc.scalar.activation(res[:], pd[3][0:1, LOGIT_COL:LOGIT_COL + B],
                                 mybir.ActivationFunctionType.Sigmoid,
                                 bias=bcol[0:1, 0:1])
            nc.sync.dma_start(out=out[:], in_=res[:])

    nc.compile()
    return nc


_NC = None


def _get_nc():
    global _NC
    if _NC is None:
        _NC = build_nc()
    return _NC


def _prep_core(feats8, refs8, rn2r, mask8, ainv, wsel, regb, c):
    """Per-core input map. feats8[l]: fp8 [B, C, HW] shard views."""
    m = {}
    f0 = feats8[0][c]                          # [32, 64, 3136] fp8
    x = np.transpose(f0, (2, 1, 0))            # [3136, 64, 32]
    x = np.pad(x, ((0, HWP0 - 3136), (0, 0), (0, 0)))
    x = x.reshape(NCH0, 128, CB0).transpose(1, 0, 2)
    m["ft0"] = np.ascontiguousarray(x)

    f1 = feats8[1][c]                          # [32, 128, 784]
    xpe = np.transpose(f1[ACT_L1:], (2, 1, 0))  # [784, 128, 8]
    xpe = np.pad(xpe, ((0, HWP1 - 784), (0, 0), (0, 0)))
    xpe = xpe.reshape(NCH1, 128, CB1).transpose(1, 0, 2)
    m["ft1pe"] = np.ascontiguousarray(xpe)
    m["f1a"] = np.ascontiguousarray(np.transpose(f1[:ACT_L1], (1, 0, 2)))

    f2 = feats8[2][c]                          # [32, 256, 196]
    x = np.transpose(f2, (1, 0, 2)).reshape(2, 128, 32 * 196)
    m["f2"] = np.ascontiguousarray(x.transpose(1, 0, 2))
    f3 = feats8[3][c]                          # [32, 512, 49]
    x = np.transpose(f3, (1, 0, 2)).reshape(4, 128, 32 * 49)
    m["f3"] = np.ascontiguousarray(x.transpose(1, 0, 2))

    for l in range(4):
        m[f"rt{l}"] = refs8[l]
    m["rn2r"] = rn2r
    m["mask8"] = mask8
    m["ainv"] = ainv
    m["wsel"] = wsel
    m["regb"] = regb
    return m


def run(trace=False, **inputs):
    nc = _get_nc()
    assert int(inputs.get("k", K)) == K
    feats8 = []
    for l in range(4):
        f = np.asarray(inputs[f"feat{l}"], dtype=np.float32)
        f = f.reshape(N_CORES, B, LAYERS[l][0], LAYERS[l][1])
        feats8.append(f.astype(F8NP))
    refs8, rn2_rows = [], np.zeros((8, RP), dtype=np.float32)
    for l, (C, HW) in enumerate(LAYERS):
        r8 = np.asarray(inputs[f"ref{l}"], dtype=np.float32).T.astype(F8NP)
        rp = np.zeros((C, RP), dtype=F8NP)
        rp[:, :R] = r8
        refs8.append(rp)                                # [C, RP] fp8
        rq = r8.astype(np.float32)
        a_rn2 = ALPHA[l] * np.sum(rq * rq, axis=0)      # [R]
        rn2_rows[2 * l, :R] = a_rn2
        rn2_rows[2 * l, R:] = 1e30
        rn2_rows[2 * l, R + 40] = QN2_SHIFT / ALPHA[l]  # global col 2040
    rn2_hi = rn2_rows.astype(ml_dtypes.bfloat16)
    lo = rn2_rows - rn2_hi.astype(np.float32)
    for l in range(4):
        rn2_hi[2 * l + 1] = lo[2 * l].astype(ml_dtypes.bfloat16)

    mask8 = np.zeros((8, 128), dtype=ml_dtypes.bfloat16)
    for l in range(4):
        mask8[2 * l, 32 * l:32 * l + 32] = -1.0
        mask8[2 * l + 1, 32 * l:32 * l + 32] = -1.0
    ainv = np.zeros((128, 1), dtype=np.float32)
    for l in range(4):
        ainv[32 * l:32 * l + 32] = 1.0 / (4.0 * ALPHA[l])
    w = np.asarray(inputs["reg_w"], dtype=np.float32).reshape(4)
    wsel = np.zeros((128, B), dtype=np.float32)
    for p in range(128):
        wsel[p, p % 32] = w[p // 32]
    regb = np.asarray(inputs["reg_b"], dtype=np.float32).reshape(1, 1)

    in_maps = [_prep_core(feats8, refs8, rn2_hi, mask8, ainv, wsel, regb, c)
               for c in range(N_CORES)]
    res = run_bass_kernel_spmd(nc, in_maps, core_ids=list(range(N_CORES)),
                               trace=trace)
    full = np.empty((N_CORES * B,), dtype=np.float32)
    for c in range(N_CORES):
        full[c * B:(c + 1) * B] = res.results[c]["out"][0, :]
    return full, res


def kernel(**inputs):
    return run(trace=False, **inputs)[0]
